# revision 20
# baseline (speedup 1.0000x reference)
"""Batched attention with K/V projection on 8 TRN2 NeuronCores.

reference (per batch b):
    keys   = states @ Wk + bk                  [S_kv, H]
    values = states @ Wv + bv                  [S_kv, H]
    scores = (query @ keys.T) / sqrt(H)        [S_q, S_kv]
    attn   = softmax(mask * scores, axis=-1)
    out    = attn @ values                     [S_q, H]

Sharding: pure data parallel — batch b -> core b (B == n_cores == 8).

Layout strategy: the TensorEngine contracts over the partition dim of both
operands, so every tensor is staged (on the host, for free) in the layout
whose contraction dim is leading:
    queryT  [H, S_q]    (pre-scaled by 1/sqrt(H) on the host)
    statesT [DIN, S_kv]
    maskT   [S_kv, S_q]
Device dataflow (all matmul inputs bf16, PSUM accumulation f32):
    keysT[h,s]    : lhsT=Wk[d,h] chunk,      rhs=statesT[d,s]      (+bk via ACT)
    values[s,h]   : lhsT=statesT[d,s] chunk, rhs=Wv[d,h]           (+bv bcast)
    scoresT[kv,q] : lhsT=keysT[h,kv] chunk,  rhs=queryT[h,q]
    E = exp(maskT * scoresT)   -- no max-subtraction needed: |mask*scores|<~7
    acc = sum of E tiles (DVE), S[q] = ones.T @ acc (single PE matmul)
    outT[h,q] = values.T @ E, normalized by 1/S[q] broadcast across partitions
Host gathers outT per core and transposes back to [B, S_q, H] f32.

Projections run dc-outer in two 8-bank PSUM waves so the PE consumes each
statesT chunk as its DMA lands instead of stalling on the full tensor.
"""

import os
import numpy as np
import ml_dtypes

B, SQ, SKV, DIN, H = 8, 2048, 2048, 1024, 512
P = 128
HC = H // P      # 4  h-chunks of 128
DC = DIN // P    # 8  d-chunks of 128
KVC = SKV // P   # 16 kv-chunks of 128
QT = SQ // 512   # 4  q-tiles of 512
ST = SKV // 512  # 4  s-tiles of 512

LAST_EXEC_NS = None
LAST_RESULTS = None
_NC = None


def _build(repeat=1):
    import contextlib
    import concourse.bacc as bacc
    import concourse.tile as tile
    import concourse.mybir as mybir

    f32 = mybir.dt.float32
    bf16 = mybir.dt.bfloat16
    Exp = mybir.ActivationFunctionType.Exp
    Ident = mybir.ActivationFunctionType.Identity

    nc = bacc.Bacc("TRN2", target_bir_lowering=False, debug=False, num_devices=8, num_swdge_queues=4)
    qT_d = nc.dram_tensor("qT", [H, SQ], bf16, kind="ExternalInput").ap()
    sT_d = nc.dram_tensor("sT", [DIN, SKV], bf16, kind="ExternalInput").ap()
    mT_d = nc.dram_tensor("mT", [SKV, SQ], bf16, kind="ExternalInput").ap()
    wk_d = nc.dram_tensor("wk", [DIN, H], bf16, kind="ExternalInput").ap()
    wv_d = nc.dram_tensor("wv", [DIN, H], bf16, kind="ExternalInput").ap()
    bk_d = nc.dram_tensor("bk", [H], f32, kind="ExternalInput").ap()
    bv_d = nc.dram_tensor("bv", [H], f32, kind="ExternalInput").ap()
    out_d = nc.dram_tensor("out", [H, SQ], f32, kind="ExternalOutput").ap()

    with tile.TileContext(nc) as tc:
        with tc.tile_pool(name="const", bufs=1) as cpool, \
             tc.tile_pool(name="big", bufs=1) as big, \
             tc.tile_pool(name="masks", bufs=12) as mpool, \
             tc.tile_pool(name="epool", bufs=32) as epool, \
             tc.tile_pool(name="tmp", bufs=4) as tpool, \
             tc.tile_pool(name="osb", bufs=4) as opool, \
             tc.tile_pool(name="ivb", bufs=2) as ipool, \
             tc.tile_pool(name="ps", bufs=8, space="PSUM") as psp, \
             (tc.For_i(0, repeat, 1) if repeat > 1 else contextlib.nullcontext()):

            # resident inputs (bf16); statesT + Wk first — they gate the PE
            wk_sb = big.tile([P, DC, H], bf16)
            wv_sb = big.tile([P, DC, H], bf16)
            st_sb = big.tile([P, DC, SKV], bf16)
            qT_sb = big.tile([P, HC, SQ], bf16)
            for dc in range(DC):
                nc.sync.dma_start(st_sb[:, dc], sT_d[dc * P:(dc + 1) * P])
                nc.sync.dma_start(wk_sb[:, dc], wk_d[dc * P:(dc + 1) * P])

            # constants (tiny; after the PE-gating loads in queue order)
            ones = cpool.tile([P, 1], f32)
            nc.any.memset(ones, 1.0)
            bk_sb = cpool.tile([P, HC], f32)
            nc.sync.dma_start(bk_sb, bk_d.rearrange("(c p) -> p c", p=P))
            bv_row = cpool.tile([1, H], f32)
            nc.sync.dma_start(bv_row, bv_d.rearrange("(o h) -> o h", o=1))
            bv_bc = cpool.tile([P, H], f32)
            nc.gpsimd.partition_broadcast(bv_bc, bv_row)

            for dc in range(DC):
                nc.sync.dma_start(wv_sb[:, dc], wv_d[dc * P:(dc + 1) * P])
            for hc in range(HC):
                nc.sync.dma_start(qT_sb[:, hc], qT_d[hc * P:(hc + 1) * P])

            kT_sb = big.tile([P, HC, SKV], bf16)
            v_sb = big.tile([P, KVC, H], bf16)

            # projections: dc-outer waves of 8 PSUM banks, so the first
            # matmuls only need statesT chunk 0 and compute overlaps the
            # statesT DMA stream.
            kjobs = [(hc, st) for hc in range(HC) for st in range(ST)]
            for wave in (kjobs[:8], kjobs[8:]):
                psums = [psp.tile([P, 512], f32, tag="ps", name=f"pj{id(wave)%97}_{i}") for i, _ in enumerate(wave)]
                for dc in range(DC):
                    for (hc, st), kp in zip(wave, psums):
                        nc.tensor.matmul(kp, wk_sb[:, dc, hc * P:(hc + 1) * P],
                                         st_sb[:, dc, st * 512:(st + 1) * 512],
                                         start=(dc == 0), stop=(dc == DC - 1))
                for (hc, st), kp in zip(wave, psums):
                    nc.scalar.activation(kT_sb[:, hc, st * 512:(st + 1) * 512],
                                         kp, Ident, bias=bk_sb[:, hc:hc + 1])
            for wave in (range(0, 8), range(8, 16)):
                psums = [psp.tile([P, 512], f32, tag="ps", name=f"pj{id(wave)%97}_{i}") for i, _ in enumerate(wave)]
                for dc in range(DC):
                    for kvc, vp in zip(wave, psums):
                        nc.tensor.matmul(vp, st_sb[:, dc, kvc * P:(kvc + 1) * P],
                                         wv_sb[:, dc],
                                         start=(dc == 0), stop=(dc == DC - 1))
                for kvc, vp in zip(wave, psums):
                    nc.vector.tensor_add(v_sb[:, kvc], vp, bv_bc)

            # attention, one 512-wide q-tile at a time
            for qt in range(QT):
                qsl = slice(qt * 512, (qt + 1) * 512)
                e_tiles = []
                acc = tpool.tile([P, 512], f32, tag="eacc")
                for kvc in range(KVC):
                    sp = psp.tile([P, 512], f32, tag="ps")
                    for hc in range(HC):
                        nc.tensor.matmul(sp, kT_sb[:, hc, kvc * P:(kvc + 1) * P],
                                         qT_sb[:, hc, qsl],
                                         start=(hc == 0), stop=(hc == HC - 1))
                    mk = mpool.tile([P, 512], bf16, tag="mask")
                    nc.sync.dma_start(mk, mT_d[kvc * P:(kvc + 1) * P, qsl])
                    tmp = tpool.tile([P, 512], f32, tag="tmp")
                    nc.vector.tensor_mul(tmp, sp, mk)
                    et = epool.tile([P, 512], bf16, tag="e")
                    nc.scalar.activation(et, tmp, Exp)
                    e_tiles.append(et)
                    # running sum of E tiles on DVE (replaces 16 PE matmuls)
                    if kvc == 0:
                        nc.vector.tensor_copy(acc, et)
                    else:
                        nc.vector.tensor_add(acc, acc, et)

                o_psums = []
                for hc in range(HC):
                    op = psp.tile([P, 512], f32, tag="ps")
                    for kvc in range(KVC):
                        nc.tensor.matmul(op, v_sb[:, kvc, hc * P:(hc + 1) * P],
                                         e_tiles[kvc],
                                         start=(kvc == 0), stop=(kvc == KVC - 1))
                    o_psums.append(op)

                # column sums over kv: single matmul on the accumulated tile
                S_ps = psp.tile([P, 512], f32, tag="ps")
                nc.tensor.matmul(S_ps[0:1, :], ones, acc, start=True, stop=True)
                invs = ipool.tile([1, 512], f32, tag="invs")
                nc.vector.reciprocal(invs, S_ps[0:1, :])
                invb = ipool.tile([P, 512], f32, tag="invb")
                nc.gpsimd.partition_broadcast(invb, invs)

                for hc in range(HC):
                    ot = opool.tile([P, 512], f32, tag="o")
                    nc.vector.tensor_mul(ot, o_psums[hc], invb)
                    nc.sync.dma_start(out_d[hc * P:(hc + 1) * P, qsl], ot)

    nc.compile()
    return nc


def kernel(query, states, mask, Wk, bk, Wv, bv):
    global LAST_EXEC_NS, LAST_RESULTS, _NC
    from concourse.bass_utils import run_bass_kernel_spmd

    if _NC is None:
        _NC = _build()

    bf = ml_dtypes.bfloat16
    scale = 1.0 / np.sqrt(np.float32(H))
    wk_b = np.ascontiguousarray(Wk.astype(bf))
    wv_b = np.ascontiguousarray(Wv.astype(bf))
    bk_f = np.ascontiguousarray(bk.astype(np.float32))
    bv_f = np.ascontiguousarray(bv.astype(np.float32))
    in_maps = []
    for b in range(B):
        in_maps.append({
            "qT": np.ascontiguousarray((query[b].T * scale).astype(bf)),
            "sT": np.ascontiguousarray(states[b].T.astype(bf)),
            "mT": np.ascontiguousarray(mask[b].T.astype(bf)),
            "wk": wk_b, "wv": wv_b, "bk": bk_f, "bv": bv_f,
        })

    trace = os.environ.get("BASS_KERNEL_TRACE", "0") not in ("", "0", "false")
    try:
        res = run_bass_kernel_spmd(_NC, in_maps, core_ids=list(range(B)), trace=trace)
    except ModuleNotFoundError:
        # NTFF profile hook unavailable in this environment; rerun untraced.
        os.environ["BASS_NEVER_TRACE"] = "1"
        res = run_bass_kernel_spmd(_NC, in_maps, core_ids=list(range(B)))
    LAST_EXEC_NS = res.exec_time_ns
    LAST_RESULTS = res
    out = np.stack([res.results[b]["out"].T for b in range(B)])
    return np.ascontiguousarray(out.astype(np.float32))


# revision 31
# speedup vs baseline: 1.0477x; 1.0477x over previous
"""Batched attention with K/V projection on 8 TRN2 NeuronCores.

reference (per batch b):
    keys   = states @ Wk + bk                  [S_kv, H]
    values = states @ Wv + bv                  [S_kv, H]
    scores = (query @ keys.T) / sqrt(H)        [S_q, S_kv]
    attn   = softmax(mask * scores, axis=-1)
    out    = attn @ values                     [S_q, H]

Sharding: pure data parallel — batch b -> core b (B == n_cores == 8).

Layout strategy: the TensorEngine contracts over the partition dim of both
operands, so every tensor is staged (on the host, for free) in the layout
whose contraction dim is leading:
    queryT  [H, S_q]    (pre-scaled by 1/sqrt(H) on the host)
    statesT [DIN, S_kv]
    maskT   [S_kv, S_q]
Device dataflow (all matmul inputs bf16, PSUM accumulation f32):
    keysT[h,s]    : lhsT=Wk[d,h] chunk,      rhs=statesT[d,s]      (+bk via ACT)
    values[s,h]   : lhsT=statesT[d,s] chunk, rhs=Wv[d,h]           (+bv bcast)
    scoresT[kv,q] : lhsT=keysT[h,kv] chunk,  rhs=queryT[h,q]
    E = exp(maskT * scoresT)   -- no max-subtraction needed: |mask*scores|<~7
    acc = sum of E tiles (DVE), S[q] = ones.T @ acc (single PE matmul)
    outT[h,q] = values.T @ E, normalized by 1/S[q] broadcast across partitions
Host gathers outT per core and transposes back to [B, S_q, H] f32.

Projections run dc-outer in two 8-bank PSUM waves so the PE consumes each
statesT chunk as its DMA lands instead of stalling on the full tensor.
"""

import os
import numpy as np
import ml_dtypes

B, SQ, SKV, DIN, H = 8, 2048, 2048, 1024, 512
P = 128
HC = H // P      # 4  h-chunks of 128
DC = DIN // P    # 8  d-chunks of 128
KVC = SKV // P   # 16 kv-chunks of 128
QT = SQ // 512   # 4  q-tiles of 512
ST = SKV // 512  # 4  s-tiles of 512

LAST_EXEC_NS = None
LAST_RESULTS = None
_NC = None


def _build(repeat=1):
    import contextlib
    import concourse.bacc as bacc
    import concourse.tile as tile
    import concourse.mybir as mybir

    f32 = mybir.dt.float32
    bf16 = mybir.dt.bfloat16
    Exp = mybir.ActivationFunctionType.Exp
    Ident = mybir.ActivationFunctionType.Identity

    nc = bacc.Bacc("TRN2", target_bir_lowering=False, debug=False, num_devices=8, num_swdge_queues=4)
    qT_d = nc.dram_tensor("qT", [H, SQ], bf16, kind="ExternalInput").ap()
    sT_d = nc.dram_tensor("sT", [DIN, SKV], bf16, kind="ExternalInput").ap()
    mT_d = nc.dram_tensor("mT", [SKV, SQ], bf16, kind="ExternalInput").ap()
    wk_d = nc.dram_tensor("wk", [DIN, H], bf16, kind="ExternalInput").ap()
    wv_d = nc.dram_tensor("wv", [DIN, H], bf16, kind="ExternalInput").ap()
    bk_d = nc.dram_tensor("bk", [H], f32, kind="ExternalInput").ap()
    bv_d = nc.dram_tensor("bv", [H], f32, kind="ExternalInput").ap()
    out_d = nc.dram_tensor("out", [H, SQ], f32, kind="ExternalOutput").ap()

    with tile.TileContext(nc) as tc:
        with tc.tile_pool(name="const", bufs=1) as cpool, \
             tc.tile_pool(name="big", bufs=1) as big, \
             tc.tile_pool(name="masks", bufs=12) as mpool, \
             tc.tile_pool(name="epool", bufs=32) as epool, \
             tc.tile_pool(name="tmp", bufs=4) as tpool, \
             tc.tile_pool(name="osb", bufs=4) as opool, \
             tc.tile_pool(name="ivb", bufs=2) as ipool, \
             tc.tile_pool(name="ps", bufs=8, space="PSUM") as psp, \
             (tc.For_i(0, repeat, 1, hint_engines=(
                  mybir.EngineType.PE, mybir.EngineType.DVE,
                  mybir.EngineType.Activation, mybir.EngineType.Pool,
                  mybir.EngineType.SP))
              if repeat > 1 else contextlib.nullcontext()):

            # resident inputs (bf16); statesT + Wk first — they gate the PE
            wk_sb = big.tile([P, DC, H], bf16)
            wv_sb = big.tile([P, DC, H], bf16)
            st_sb = big.tile([P, DC, SKV], bf16)
            qT_sb = big.tile([P, HC, SQ], bf16)
            # chunk 0 split per s-tile: the first wave's dc=0 matmuls only
            # need wk chunk 0 plus one 512-column piece of statesT chunk 0,
            # so the PE starts ~1us earlier.
            nc.sync.dma_start(wk_sb[:, 0], wk_d[0:P])
            for st in range(ST):
                nc.sync.dma_start(st_sb[:, 0, st * 512:(st + 1) * 512],
                                  sT_d[0:P, st * 512:(st + 1) * 512])
            for dc in range(1, DC):
                nc.sync.dma_start(st_sb[:, dc], sT_d[dc * P:(dc + 1) * P])
                nc.sync.dma_start(wk_sb[:, dc], wk_d[dc * P:(dc + 1) * P])

            # constants (tiny; after the PE-gating loads in queue order)
            ones = cpool.tile([P, 1], bf16)
            nc.any.memset(ones, 1.0)
            bk_sb = cpool.tile([P, HC], f32)
            nc.sync.dma_start(bk_sb, bk_d.rearrange("(c p) -> p c", p=P))
            bv_row = cpool.tile([1, H], f32)
            nc.sync.dma_start(bv_row, bv_d.rearrange("(o h) -> o h", o=1))
            bv_bc = cpool.tile([P, H], f32)
            nc.gpsimd.partition_broadcast(bv_bc, bv_row)

            for dc in range(DC):
                nc.sync.dma_start(wv_sb[:, dc], wv_d[dc * P:(dc + 1) * P])
            for hc in range(HC):
                nc.sync.dma_start(qT_sb[:, hc], qT_d[hc * P:(hc + 1) * P])

            kT_sb = big.tile([P, HC, SKV], bf16)
            v_sb = big.tile([P, KVC, H], bf16)

            # projections: dc-outer waves of 8 PSUM banks, so the first
            # matmuls only need statesT chunk 0 and compute overlaps the
            # statesT DMA stream.
            # st-major order: consecutive matmuls in a wave then use
            # different lhsT (wk[dc,hc] alternates), letting LDWEIGHTS of the
            # next MM overlap the in-flight MM via the background buffer.
            kjobs = [(hc, st) for st in range(ST) for hc in range(HC)]
            for wave in (kjobs[:8], kjobs[8:]):
                psums = [psp.tile([P, 512], f32, tag="ps", name=f"pj{id(wave)%97}_{i}") for i, _ in enumerate(wave)]
                for dc in range(DC):
                    for (hc, st), kp in zip(wave, psums):
                        nc.tensor.matmul(kp, wk_sb[:, dc, hc * P:(hc + 1) * P],
                                         st_sb[:, dc, st * 512:(st + 1) * 512],
                                         start=(dc == 0), stop=(dc == DC - 1))
                for (hc, st), kp in zip(wave, psums):
                    nc.scalar.activation(kT_sb[:, hc, st * 512:(st + 1) * 512],
                                         kp, Ident, bias=bk_sb[:, hc:hc + 1])
            for wave in (range(0, 8), range(8, 16)):
                psums = [psp.tile([P, 512], f32, tag="ps", name=f"pj{id(wave)%97}_{i}") for i, _ in enumerate(wave)]
                for dc in range(DC):
                    for kvc, vp in zip(wave, psums):
                        nc.tensor.matmul(vp, st_sb[:, dc, kvc * P:(kvc + 1) * P],
                                         wv_sb[:, dc],
                                         start=(dc == 0), stop=(dc == DC - 1))
                for kvc, vp in zip(wave, psums):
                    nc.vector.tensor_add(v_sb[:, kvc], vp, bv_bc)

            # attention, one 512-wide q-tile at a time
            for qt in range(QT):
                qsl = slice(qt * 512, (qt + 1) * 512)
                e_tiles = []
                acc = tpool.tile([P, 512], f32, tag="eacc")
                for kvc in range(KVC):
                    sp = psp.tile([P, 512], f32, tag="ps")
                    for hc in range(HC):
                        nc.tensor.matmul(sp, kT_sb[:, hc, kvc * P:(kvc + 1) * P],
                                         qT_sb[:, hc, qsl],
                                         start=(hc == 0), stop=(hc == HC - 1))
                    mk = mpool.tile([P, 512], bf16, tag="mask")
                    nc.sync.dma_start(mk, mT_d[kvc * P:(kvc + 1) * P, qsl])
                    tmp = tpool.tile([P, 512], f32, tag="tmp")
                    nc.vector.tensor_mul(tmp, sp, mk)
                    et = epool.tile([P, 512], bf16, tag="e")
                    nc.scalar.activation(et, tmp, Exp)
                    e_tiles.append(et)
                    # running sum of E tiles on DVE (replaces 16 PE matmuls)
                    if kvc == 0:
                        nc.vector.tensor_copy(acc, et)
                    else:
                        nc.vector.tensor_add(acc, acc, et)

                # column sums over kv: single bf16 matmul on the accumulated
                # tile (per-partition bf16 rounding of acc averages out over
                # the 128-partition f32 PSUM sum: ~0.03% error on S)
                acc_bf = tpool.tile([P, 512], bf16, tag="eaccb")
                nc.vector.tensor_copy(acc_bf, acc)

                o_psums = []
                for hc in range(HC):
                    op = psp.tile([P, 512], f32, tag="ps")
                    for kvc in range(KVC):
                        nc.tensor.matmul(op, v_sb[:, kvc, hc * P:(hc + 1) * P],
                                         e_tiles[kvc],
                                         start=(kvc == 0), stop=(kvc == KVC - 1))
                    o_psums.append(op)

                S_ps = psp.tile([P, 512], f32, tag="ps")
                nc.tensor.matmul(S_ps[0:1, :], ones, acc_bf, start=True, stop=True)
                invs = ipool.tile([1, 512], f32, tag="invs")
                nc.vector.reciprocal(invs, S_ps[0:1, :])
                invb = ipool.tile([P, 512], f32, tag="invb")
                nc.gpsimd.partition_broadcast(invb, invs)

                for hc in range(HC):
                    ot = opool.tile([P, 512], f32, tag="o")
                    nc.vector.tensor_mul(ot, o_psums[hc], invb)
                    nc.sync.dma_start(out_d[hc * P:(hc + 1) * P, qsl], ot)

    nc.compile()
    return nc


def kernel(query, states, mask, Wk, bk, Wv, bv):
    global LAST_EXEC_NS, LAST_RESULTS, _NC
    from concourse.bass_utils import run_bass_kernel_spmd

    if _NC is None:
        _NC = _build()

    query = np.asarray(query)
    states = np.asarray(states)
    mask = np.asarray(mask)
    Wk, bk, Wv, bv = (np.asarray(x) for x in (Wk, bk, Wv, bv))
    bf = ml_dtypes.bfloat16
    scale = 1.0 / np.sqrt(np.float32(H))
    wk_b = np.ascontiguousarray(Wk.astype(bf))
    wv_b = np.ascontiguousarray(Wv.astype(bf))
    bk_f = np.ascontiguousarray(bk.astype(np.float32))
    bv_f = np.ascontiguousarray(bv.astype(np.float32))
    in_maps = []
    for b in range(B):
        in_maps.append({
            "qT": np.ascontiguousarray((query[b].T * scale).astype(bf)),
            "sT": np.ascontiguousarray(states[b].T.astype(bf)),
            "mT": np.ascontiguousarray(mask[b].T.astype(bf)),
            "wk": wk_b, "wv": wv_b, "bk": bk_f, "bv": bv_f,
        })

    trace = os.environ.get("BASS_KERNEL_TRACE", "0") not in ("", "0", "false")
    try:
        res = run_bass_kernel_spmd(_NC, in_maps, core_ids=list(range(B)), trace=trace)
    except ModuleNotFoundError:
        # NTFF profile hook unavailable in this environment; rerun untraced.
        os.environ["BASS_NEVER_TRACE"] = "1"
        res = run_bass_kernel_spmd(_NC, in_maps, core_ids=list(range(B)))
    LAST_EXEC_NS = res.exec_time_ns
    LAST_RESULTS = res
    out = np.stack([res.results[b]["out"].T for b in range(B)])
    return np.ascontiguousarray(out.astype(np.float32))
